# revision 34
# baseline (speedup 1.0000x reference)
"""EntityAttentionLayer on 8 Trainium2 NeuronCores (Bass/Tile).

Reference computation (per batch b of 1024):
    qkv = entities @ W_in.T            # [128 ents, 3*512]
    q (first 32 ents), k, v -> 8 heads x 64
    logits = q k^T / 8, masked by pre_mask (True = masked out)
    w = softmax(logits), fully-masked rows -> 0
    out = (w v) @ W_out.T + b_out, zeroed where post_mask

Sharding: data-parallel over batch, 128 batches per core.

Per-core kernel design (v2):
  - The PE streams ~1 column/cycle at 2.4 GHz; budget = streamed columns:
    49152 per 8-batch iteration (QKV 36864, logits 4096 block-diag 2-heads/
    matmul w/ tile_position column packing, transpose 2048, attn@v 2048,
    out-proj 4096).
  - softmax: exp on scalar (unmasked logits), then ONE vector
    tensor_tensor_reduce per (batch, head-pair-group) applies the 0/1 keep
    mask multiplicatively AND produces the row sums; reciprocal per group,
    per-(bs,g) tensor_scalar normalize. This keeps the vector engine well
    under the PE budget (the v1 additive-mask PSUM add was 2x1220ns/chunk).
  - w is PE-transposed per (batch, group); attn@v contracts over entities
    with 2-head column packing; out-projection 4-ki accumulation with fused
    (bias add) x (post-mask mul) epilogue.
  - scheduling: per iteration the 80 QKV matmuls (16 q, 32 k, 32 v) are
    zipped 1:1 with the 80 attention PE ops of the previous iteration
    (16 logit pairs, 16 transposes, 32 attn@v pairs, 16 out-proj), so every
    small-matmul weight load hides under a 512-col k/v stream and the
    vector softmax chain runs ~2.5us ahead of its PE consumers.
  - startup: W_in section DMAs split per-128-col slice and xta per-ki so the
    first q matmul starts as soon as ~192KB has landed; identity/qbd
    memsets issue before any compute.
  - drain: the last iteration defers k(g2=1) and v(b4..7) to act as PE
    fillers for its own attention; out-projection runs per-4-batch subchunk
    with quarter output DMAs so the tail after the last matmul is short.
  - PSUM: 4-buffer ring for projection matmuls, 2 logits banks, 1 transpose
    bank, 1 bank shared by attn@v and out-projection accumulators.
"""
import sys

sys.path.insert(0, "/opt/trn_rl_repo")

import numpy as np
import ml_dtypes

BS, NE, IN_DIM = 1024, 128, 512
EMBED, OUT_DIM = 512, 512
N_HEADS, N_AGENTS = 8, 32
HEAD_DIM = EMBED // N_HEADS  # 64
N_CORES = 8
USE_TTR = False  # fuse mask-mult + row-sum into one tensor_tensor_reduce


def build_nc(b_core: int):
    """Build the per-core Bass program for b_core batches (b_core % 8 == 0)."""
    import concourse.bass as bass
    import concourse.tile as tile
    from concourse import bacc, mybir
    from concourse.masks import make_identity

    F32 = mybir.dt.float32
    BF16 = mybir.dt.bfloat16
    Exp = mybir.ActivationFunctionType.Exp
    Alu = mybir.AluOpType

    assert b_core % 8 == 0
    n_iter = b_core // 8

    nc = bacc.Bacc("TRN2", target_bir_lowering=False, debug=False)

    xt_d = nc.declare_dram_parameter("xt", [b_core, IN_DIM, NE], BF16, isOutput=False)
    xta_d = nc.declare_dram_parameter("xta", [4, 128, b_core, N_AGENTS], BF16, isOutput=False)
    wi_d = nc.declare_dram_parameter("wi", [IN_DIM, 3 * EMBED], BF16, isOutput=False)
    wo_d = nc.declare_dram_parameter("wo", [EMBED, OUT_DIM], BF16, isOutput=False)
    keep_d = nc.declare_dram_parameter("keep", [b_core, N_AGENTS, NE], BF16, isOutput=False)
    pkeep_d = nc.declare_dram_parameter("pkeep", [b_core, N_AGENTS], F32, isOutput=False)
    bias_d = nc.declare_dram_parameter("bias", [OUT_DIM], F32, isOutput=False)
    out_d = nc.declare_dram_parameter("out", [OUT_DIM, b_core, N_AGENTS], BF16, isOutput=True)

    AP = bass.AP

    def dram_ap(handle, offset, ap):
        base = handle[:]
        return AP(tensor=base.tensor, offset=offset, ap=ap)

    with tile.TileContext(nc) as tc:
        with (
            tc.tile_pool(name="const", bufs=1) as constp,
            tc.tile_pool(name="ins", bufs=2) as insp,
            tc.tile_pool(name="mid", bufs=2) as midp,
            tc.tile_pool(name="attn", bufs=2) as attnp,
            tc.tile_pool(name="outs", bufs=2) as outsp,
            tc.tile_pool(name="ps_mm", bufs=4, space="PSUM") as ps_mm,
            tc.tile_pool(name="ps_lg", bufs=1, space="PSUM") as ps_lg,
            tc.tile_pool(name="ps_wt", bufs=1, space="PSUM") as ps_wt,
            tc.tile_pool(name="ps_at", bufs=1, space="PSUM") as ps_at,
        ):
            # ---- constants ----
            wi_sb = constp.tile([128, 4, 3 * EMBED], BF16, name="wi_sb", tag="wi_sb")
            wo_sb = constp.tile([128, 4, OUT_DIM], BF16)
            bias_sb = constp.tile([128, 4], F32)
            ident = constp.tile([128, 128], BF16)
            # block-diagonal q operands, double-buffered manually; off-diagonal
            # zero blocks are written once and never touched again
            qbd = [
                constp.tile([128, 4, 8, 64], BF16, name=f"qbd_{i}", tag=f"qbd_{i}")
                for i in range(2)
            ]

            def emit_wi_section(sec, mo=None):
                # DMA one section (0=q, 1=k, 2=v) of W_in (all 4 ki); per-mo
                # 128-col slices let iter-0 compute start as data arrives
                c0 = sec * EMBED + (0 if mo is None else mo * 128)
                nc2 = EMBED if mo is None else 128
                nc.sync.dma_start(
                    out=wi_sb[:, :, c0 : c0 + nc2],
                    in_=dram_ap(
                        wi_d, c0,
                        [[3 * EMBED, 128], [128 * 3 * EMBED, 4], [1, nc2]],
                    ),
                )

            def emit_early_consts():
                make_identity(nc, ident)
                for i in range(2):
                    nc.gpsimd.memset(qbd[i][0:64, :, :, 32:64], 0.0)
                    nc.gpsimd.memset(qbd[i][64:128, :, :, 0:32], 0.0)

            def emit_late_consts():
                nc.sync.dma_start(
                    out=wo_sb,
                    in_=dram_ap(wo_d, 0, [[OUT_DIM, 128], [128 * OUT_DIM, 4], [1, OUT_DIM]]),
                )
                nc.sync.dma_start(out=bias_sb, in_=dram_ap(bias_d, 0, [[1, 128], [128, 4]]))

            def emit_xta(st, it, ki=None):
                b0 = it * 8
                k0, nk = (0, 4) if ki is None else (ki, 1)
                nc.gpsimd.dma_start(
                    out=st["xta"][:, k0 : k0 + nk, :, :],
                    in_=dram_ap(
                        xta_d,
                        k0 * 128 * b_core * N_AGENTS + b0 * N_AGENTS,
                        [[b_core * N_AGENTS, 128], [128 * b_core * N_AGENTS, nk],
                         [N_AGENTS, 8], [1, N_AGENTS]],
                    ),
                )

            def emit_xt_chunk(st, it, g2, ki):
                b0 = it * 8
                nc.sync.dma_start(
                    out=st["xt"][:, ki, g2 * 4 : (g2 + 1) * 4, :],
                    in_=dram_ap(
                        xt_d,
                        (b0 + g2 * 4) * IN_DIM * NE + ki * 128 * NE,
                        [[NE, 128], [IN_DIM * NE, 4], [1, NE]],
                    ),
                )

            def emit_masks(st, it):
                b0 = it * 8
                # 0/1 keep mask, replicated over the 4 head-pair partition groups
                for cg in range(4):
                    nc.gpsimd.dma_start(
                        out=st["keep"][cg * 32 : (cg + 1) * 32, :, :],
                        in_=dram_ap(
                            keep_d,
                            b0 * N_AGENTS * NE,
                            [[NE, 32], [N_AGENTS * NE, 8], [1, NE]],
                        ),
                    )
                nc.gpsimd.dma_start(
                    out=st["pkeep"],
                    in_=dram_ap(pkeep_d, b0 * N_AGENTS, [[0, 128], [N_AGENTS, 8], [1, N_AGENTS]]),
                )

            def new_state(it):
                st = {"it": it, "qbd": qbd[it % 2]}
                st["xta"] = insp.tile([128, 4, 8, N_AGENTS], BF16, name="xta_sb", tag="xta_sb")
                st["xt"] = insp.tile([128, 4, 8, NE], BF16, name="xt_sb", tag="xt_sb")
                st["keep"] = insp.tile([128, 8, NE], BF16, name="keep_bc", tag="keep_bc")
                # bufs=3: the deferred out-projection reads pkeep(N) during
                # iteration N+2, after iteration N+2's input DMAs are issued
                st["pkeep"] = insp.tile(
                    [128, 8, N_AGENTS], F32, name="pkeep_bc", tag="pkeep_bc", bufs=3
                )
                st["kt"] = midp.tile([128, 4, 8, NE], BF16, name="kt_sb", tag="kt_sb")
                st["vt"] = midp.tile([128, 8, EMBED], BF16, name="vt_sb", tag="vt_sb")
                return st

            def emit_inputs(it):
                """Issue a steady-state iteration's input DMAs."""
                st = new_state(it)
                emit_xta(st, it)
                for g2 in range(2):
                    for ki in range(4):
                        emit_xt_chunk(st, it, g2, ki)
                emit_masks(st, it)
                return st

            # ---- per-iteration fine-grained PE op lists ----

            def q_unit(st, mo):
                """4 q matmuls; returns the PSUM evacuation closure (emitted
                a couple of units later so the copy never heads the scalar/
                vector queue before its data is ready)."""
                q_ps = ps_mm.tile([128, 8, N_AGENTS], F32, tag="mm", name="q_ps")
                for ki in range(4):
                    nc.tensor.matmul(
                        q_ps,
                        wi_sb[:, ki, mo * 128 : (mo + 1) * 128],
                        st["xta"][:, ki, :, :],
                        start=(ki == 0),
                        stop=(ki == 3),
                    )
                qb = st["qbd"]

                def cp():
                    nc.scalar.copy(out=qb[0:64, mo, :, 0:32], in_=q_ps[0:64, :, :])
                    nc.vector.tensor_copy(out=qb[64:128, mo, :, 32:64], in_=q_ps[64:128, :, :])

                return cp

            def k_unit(st, mo, g2):
                k_ps = ps_mm.tile([128, 4, NE], F32, tag="mm", name="k_ps")
                for ki in range(4):
                    nc.tensor.matmul(
                        k_ps,
                        wi_sb[:, ki, EMBED + mo * 128 : EMBED + (mo + 1) * 128],
                        st["xt"][:, ki, g2 * 4 : (g2 + 1) * 4, :],
                        start=(ki == 0),
                        stop=(ki == 3),
                    )

                def cp():
                    if g2 == 0:
                        nc.vector.tensor_copy(
                            out=st["kt"][:, mo, g2 * 4 : (g2 + 1) * 4, :], in_=k_ps
                        )
                    else:
                        nc.scalar.copy(out=st["kt"][:, mo, g2 * 4 : (g2 + 1) * 4, :], in_=k_ps)

                return cp

            def v_unit(st, b):
                v_ps = ps_mm.tile([128, EMBED], F32, tag="mm", name="v_ps")
                for ki in range(4):
                    nc.tensor.matmul(
                        v_ps,
                        st["xt"][:, ki, b, :],
                        wi_sb[:, ki, 2 * EMBED : 3 * EMBED],
                        start=(ki == 0),
                        stop=(ki == 3),
                    )

                def cp():
                    nc.scalar.copy(out=st["vt"][:, b, :], in_=v_ps)

                return cp

            def qkv_units(st, last=False):
                """Unit closures (4 same-weights matmuls each) in v1 order.
                last=True: returns (main12, ktail4, vtail4) with k(g2=1) and
                v(b4..7) deferred for use as drain fillers."""
                qs = [lambda mo=mo: q_unit(st, mo) for mo in range(4)]
                if last:
                    main = qs + [
                        lambda mo=mo: k_unit(st, mo, 0) for mo in range(4)
                    ] + [lambda b=b: v_unit(st, b) for b in range(4)]
                    ktail = [lambda mo=mo: k_unit(st, mo, 1) for mo in range(4)]
                    vtail = [lambda b=b: v_unit(st, b) for b in range(4, 8)]
                    return main, ktail, vtail
                units = list(qs)
                for mo in range(4):
                    units.append(lambda mo=mo: k_unit(st, mo, 0))
                    units.append(lambda mo=mo: k_unit(st, mo, 1))
                    units.append(lambda b=2 * mo: v_unit(st, b))
                    units.append(lambda b=2 * mo + 1: v_unit(st, b))
                return units

            def softmax_ops(st, sc):
                """8 PE closures (logit pairs, g-major) with the scalar/vector
                softmax chain attached: exp per g, then per-(bs,g) mask-mult +
                row-sum (one tensor_tensor_reduce), reciprocal per g,
                per-(bs,g) normalize."""
                qb, kt = st["qbd"], st["kt"]
                lg = [
                    ps_lg.tile([128, 4, NE], F32, tag="lg0", name="lg0"),
                    ps_lg.tile([128, 4, NE], F32, tag="lg1", name="lg1"),
                ]  # [(hp2, h2, a), bs, e] for head-pair-group g = 0, 1
                we = attnp.tile([128, 4, 2, NE], BF16, name="we", tag="we")
                sums = attnp.tile([128, 8], F32, name="sums", tag="sums")
                rcp = attnp.tile([128, 8], F32, name="rcp", tag="rcp")
                wn = attnp.tile([128, 4, 2, NE], BF16, name="wn", tag="wn")
                st[f"wn{sc}"] = wn

                def lg_pair(bs, g):
                    b = sc * 4 + bs
                    for hp2 in range(2):
                        hp = g * 2 + hp2
                        nc.tensor.matmul(
                            lg[g][hp2 * 64 : (hp2 + 1) * 64, bs, :],
                            qb[:, hp, b, :],
                            kt[:, hp, b, :],
                            start=True,
                            stop=True,
                            tile_position=(0, hp2 * 64),
                        )

                def post_g(g):
                    # exp of the whole group's logits (PSUM f32 -> SBUF bf16)
                    nc.scalar.activation(out=we[:, :, g, :], in_=lg[g], func=Exp)
                    # multiplicative 0/1 keep mask + row sums (one op each per g)
                    nc.vector.tensor_mul(
                        wn[:, :, g, :],
                        we[:, :, g, :],
                        st["keep"][:, sc * 4 : (sc + 1) * 4, :],
                    )
                    nc.vector.reduce_sum(
                        sums[:, g * 4 : (g + 1) * 4],
                        wn[:, :, g, :],
                        axis=mybir.AxisListType.X,
                    )
                    nc.vector.reciprocal_approx_fast(
                        out=rcp[:, g * 4 : (g + 1) * 4], in_=sums[:, g * 4 : (g + 1) * 4]
                    )
                    for bs in range(4):
                        nc.vector.tensor_scalar_mul(
                            wn[:, bs, g, :],
                            wn[:, bs, g, :],
                            rcp[:, g * 4 + bs : g * 4 + bs + 1],
                        )

                ops = []
                for g in range(2):
                    for bs in range(4):
                        if bs == 3:
                            ops.append(lambda bs=bs, g=g: (lg_pair(bs, g), post_g(g)))
                        else:
                            ops.append(lambda bs=bs, g=g: lg_pair(bs, g))
                return ops

            def attnv_ops(st, sc, attn_sb):
                """8 transpose closures (g-major, per-(bs,g) PSUM->SBUF copy
                attached) + 16 attn@v pair closures (bs-major, per-bs
                attn_sb copy attached)."""
                vt = st["vt"]
                wt_ps = ps_wt.tile([128, 4, 2, NE], BF16, name="wt_ps")  # [e, bs, g, (hp2,h2,a)]
                wt_sb = attnp.tile([128, 4, 2, NE], BF16, name="wt_sb", tag="wt_sb")
                at_ps = ps_at.tile([128, 4, 4, N_AGENTS], F32, name="at_ps", tag="atop")

                def t_op(bs, g):
                    wn = st[f"wn{sc}"]
                    nc.tensor.transpose(wt_ps[:, bs, g, :], wn[:, bs, g, :], ident)
                    if bs == 3:
                        # scalar: its queue drains right after the exps, so
                        # the attn@v leads aren't gated on the vector backlog
                        nc.scalar.copy(out=wt_sb[:, :, g, :], in_=wt_ps[:, :, g, :])

                def av_pair(bs, hp):
                    b = sc * 4 + bs
                    g, hp2 = hp // 2, hp % 2
                    for h2 in range(2):
                        h = hp * 2 + h2
                        nc.tensor.matmul(
                            at_ps[h2 * 64 : h2 * 64 + 64, bs, hp, :],
                            vt[:, b, h * 64 : (h + 1) * 64],
                            wt_sb[:, bs, g, hp2 * 64 + h2 * 32 : hp2 * 64 + (h2 + 1) * 32],
                            start=True,
                            stop=True,
                            tile_position=(0, h2 * 64),
                        )
                    if hp == 3:
                        nc.scalar.copy(
                            out=attn_sb[:, sc * 4 + bs, :, :], in_=at_ps[:, bs, :, :]
                        )

                ops = [lambda bs=bs, g=g: t_op(bs, g) for g in range(2) for bs in range(4)]
                ops += [lambda bs=bs, hp=hp: av_pair(bs, hp) for bs in range(4) for hp in range(4)]
                return ops

            def emit_outdma(st, out_sb, mh, sc=None, queue=None):
                # stream a 256-feature half (or quarter, if sc given) out
                b0, nbs = (0, 8) if sc is None else (sc * 4, 4)
                q = queue if queue is not None else nc.gpsimd
                q.dma_start(
                    out=dram_ap(
                        out_d,
                        mh * 2 * 128 * b_core * N_AGENTS + (st["it"] * 8 + b0) * N_AGENTS,
                        [[b_core * N_AGENTS, 128],
                         [128 * b_core * N_AGENTS, 2],
                         [N_AGENTS, nbs],
                         [1, N_AGENTS]],
                    ),
                    in_=out_sb[:, mh * 2 : mh * 2 + 2, b0 : b0 + nbs, :],
                )

            def outproj_ops(st, attn_sb, out_sb, sc, dma, queue=None):
                """Closures for the out-projection: 16 matmuls with the
                (bias add) x (post-mask mul) epilogue + output DMA attached."""
                bs0, nbs = (0, 8) if sc is None else (sc * 4, 4)
                # allocate cells at build time so the bank rotation order is
                # [at_ps(N), cells(N), at_ps(N+1), ...] — a full iteration of
                # slack on each handoff even though the op block runs deferred
                cell = {
                    mh: ps_at.tile([128, 2, nbs, N_AGENTS], F32, name="op_ps", tag="atop")
                    for mh in range(2)
                }

                def mm(mh, m2, ki2):
                    mo2 = mh * 2 + m2
                    nc.tensor.matmul(
                        cell[mh][:, m2, :, :],
                        wo_sb[:, ki2, mo2 * 128 : (mo2 + 1) * 128],
                        attn_sb[:, bs0 : bs0 + nbs, ki2, :],
                        start=(ki2 == 0),
                        stop=(ki2 == 3),
                    )

                def stt(mh):
                    for m2 in range(2):
                        mo2 = mh * 2 + m2
                        nc.vector.scalar_tensor_tensor(
                            out=out_sb[:, mo2, bs0 : bs0 + nbs, :],
                            in0=cell[mh][:, m2, :, :],
                            scalar=bias_sb[:, mo2 : mo2 + 1],
                            in1=st["pkeep"][:, bs0 : bs0 + nbs, :],
                            op0=Alu.add,
                            op1=Alu.mult,
                        )
                    if dma:
                        emit_outdma(st, out_sb, mh, sc=sc, queue=queue)

                # the epilogue stt closures come after ALL matmuls so the
                # vector queue never blocks on a not-yet-computed cell
                ops = [
                    lambda mh=mh, m2=m2, ki2=ki2: mm(mh, m2, ki2)
                    for mh in range(2) for m2 in range(2) for ki2 in range(4)
                ]
                ops.append(lambda: stt(0))
                ops.append(lambda: stt(1))
                return ops

            def attn_ops(st):
                """Fine PE closures for one iteration's attention: the
                softmax/transpose/attn@v front (64 ops) and the out-projection
                block (18 ops), which the caller defers by one iteration."""
                attn_sb = outsp.tile([128, 8, 4, N_AGENTS], BF16, name="attn_sb", tag="attn_sb")
                out_sb = outsp.tile([128, 4, 8, N_AGENTS], BF16, name="out_sb", tag="out_sb")
                st["attn_sb"] = attn_sb
                st["out_sb"] = out_sb
                front = softmax_ops(st, 0) + softmax_ops(st, 1)
                front += attnv_ops(st, 0, attn_sb)
                front += attnv_ops(st, 1, attn_sb)
                opl = outproj_ops(st, attn_sb, out_sb, None, dma=True)
                return front, opl

            # ---- head: iteration 0 with DMA-overlapped startup ----
            st0 = new_state(0)
            emit_wi_section(0, mo=0)
            emit_xta(st0, 0, ki=0)
            emit_xta(st0, 0, ki=1)
            emit_xta(st0, 0, ki=2)
            emit_xta(st0, 0, ki=3)
            emit_early_consts()
            emit_wi_section(0, mo=1)
            emit_wi_section(0, mo=2)
            emit_wi_section(0, mo=3)
            for ki in range(4):
                emit_xt_chunk(st0, 0, 0, ki)
                emit_wi_section(1, mo=ki)
            for ki in range(4):
                emit_xt_chunk(st0, 0, 1, ki)
                emit_wi_section(2, mo=ki)
            emit_masks(st0, 0)
            emit_late_consts()

            # pops per unit: logits pre-pop before the units (their vector
            # softmax chain then runs ahead of the q-copy burst); transposes
            # from u4 (wn ready ~u3), attn@v after the wt copies, out-proj
            # once sc1's attn_sb lands.
            STEADY_POPS = [4] * 19 + [6]
            LAST_POPS = [4, 4, 4, 4, 8, 8, 8, 8, 8, 8, 8, 10]

            # evacuation copies are flushed 2 units after their matmuls so
            # they never head an engine queue before their data is ready;
            # the deque carries across iterations
            useq = [0]
            pend = []

            def run_unit(u):
                cp = u()
                while pend and pend[0][0] <= useq[0] - 1:
                    pend.pop(0)[1]()
                pend.append((useq[0], cp))
                useq[0] += 1

            def flush_pend():
                while pend:
                    pend.pop(0)[1]()

            def weave(units, aops, pops):
                rest = list(aops)
                for u, np_ in zip(units, pops):
                    run_unit(u)
                    for _ in range(np_):
                        if rest:
                            rest.pop(0)()
                for u in rest:
                    u()

            prev = None
            for it in range(n_iter - 1):
                st = st0 if it == 0 else emit_inputs(it)
                units = qkv_units(st)
                if prev is not None:
                    front, opl = attn_ops(prev)
                    weave(units, front + opl, STEADY_POPS)
                else:
                    for u in units:
                        run_unit(u)
                prev = st

            # ---- last iteration: weave prev's attention with main qkv, then
            # self-interleave its own attention with the deferred k/v fillers
            st = emit_inputs(n_iter - 1) if n_iter > 1 else st0
            main, ktail, vtail = qkv_units(st, last=True)
            if prev is not None:
                front, opl = attn_ops(prev)
                weave(main, front + opl, LAST_POPS)
            else:
                for u in main:
                    run_unit(u)

            attn_sb = outsp.tile([128, 8, 4, N_AGENTS], BF16, name="attn_sb", tag="attn_sb")
            out_sb = outsp.tile([128, 4, 8, N_AGENTS], BF16, name="out_sb", tag="out_sb")
            st["attn_sb"] = attn_sb
            sm0 = softmax_ops(st, 0)
            for u in ktail:  # 4 k units; 8 logit pairs woven in
                run_unit(u)
                sm0.pop(0)()
                sm0.pop(0)()
            av0 = attnv_ops(st, 0, attn_sb)
            for i, u in enumerate(vtail):  # 4 v units; 24 attn ops woven in
                run_unit(u)
                for _ in range(6):
                    av0.pop(0)()
            flush_pend()
            sm1 = softmax_ops(st, 1)
            op0 = outproj_ops(st, attn_sb, out_sb, 0, dma=True)
            for i, u in enumerate(op0):  # 16 op matmuls; 8 logit pairs woven in
                u()
                if i % 2 == 1 and sm1:
                    sm1.pop(0)()
            for u in attnv_ops(st, 1, attn_sb):
                u()
            for u in outproj_ops(st, attn_sb, out_sb, 1, dma=True, queue=nc.sync):
                u()

    nc.compile()
    return nc


def _prep_core_inputs(ents, keep, pkeep, wi, wo, bias):
    """Host-side layout prep for one core's batch shard."""
    b_core = ents.shape[0]
    xt = np.ascontiguousarray(ents.transpose(0, 2, 1))  # [b, in, e]
    xta = np.ascontiguousarray(
        ents[:, :N_AGENTS, :].transpose(2, 0, 1)
    ).reshape(4, 128, b_core, N_AGENTS)
    return {
        "xt": xt,
        "xta": xta,
        "wi": wi,
        "wo": wo,
        "keep": keep,
        "pkeep": pkeep,
        "bias": bias,
    }


def run(entities, pre_mask, post_mask, W_in, W_out, b_out, trace=False):
    """Shard, run on 8 cores, gather. Returns (out, BassKernelResults)."""
    from concourse.bass_utils import run_bass_kernel_spmd

    bs = entities.shape[0]
    b_core = bs // N_CORES
    entities = np.asarray(entities, dtype=np.float32).astype(ml_dtypes.bfloat16)
    keep = (~np.asarray(pre_mask)).astype(ml_dtypes.bfloat16)
    pkeep = (~np.asarray(post_mask)).astype(np.float32)
    wi_f = np.ascontiguousarray(np.asarray(W_in, dtype=np.float32).T)
    wi_f[:, :EMBED] *= np.float32(0.125)  # fold the 1/sqrt(head_dim) into W_q
    wi = wi_f.astype(ml_dtypes.bfloat16)
    wo = np.ascontiguousarray(np.asarray(W_out, dtype=np.float32).T).astype(ml_dtypes.bfloat16)
    bias = np.asarray(b_out, dtype=np.float32)

    nc = build_nc(b_core)
    in_maps = [
        _prep_core_inputs(
            entities[c * b_core : (c + 1) * b_core],
            keep[c * b_core : (c + 1) * b_core],
            pkeep[c * b_core : (c + 1) * b_core],
            wi, wo, bias,
        )
        for c in range(N_CORES)
    ]
    res = run_bass_kernel_spmd(nc, in_maps, list(range(N_CORES)), trace=trace)
    out = np.empty((bs, N_AGENTS, OUT_DIM), dtype=np.float32)
    for c in range(N_CORES):
        out[c * b_core : (c + 1) * b_core] = (
            res.results[c]["out"].astype(np.float32).transpose(1, 2, 0)
        )
    return out, res


def kernel(entities, pre_mask, post_mask, W_in, W_out, b_out):
    out, _ = run(entities, pre_mask, post_mask, W_in, W_out, b_out, trace=False)
    return out


# revision 40
# speedup vs baseline: 1.1986x; 1.1986x over previous
"""EntityAttentionLayer on 8 Trainium2 NeuronCores (Bass/Tile).

Reference computation (per batch b of 1024):
    qkv = entities @ W_in.T            # [128 ents, 3*512]
    q (first 32 ents), k, v -> 8 heads x 64
    logits = q k^T / 8, masked by pre_mask (True = masked out)
    w = softmax(logits), fully-masked rows -> 0
    out = (w v) @ W_out.T + b_out, zeroed where post_mask

Sharding: data-parallel over batch, 128 batches per core.

Per-core kernel design (v2):
  - The PE streams ~1 column/cycle at 2.4 GHz; budget = streamed columns:
    49152 per 8-batch iteration (QKV 36864, logits 4096 block-diag 2-heads/
    matmul w/ tile_position column packing, transpose 2048, attn@v 2048,
    out-proj 4096).
  - softmax: exp on scalar (unmasked logits), then ONE vector
    tensor_tensor_reduce per (batch, head-pair-group) applies the 0/1 keep
    mask multiplicatively AND produces the row sums; reciprocal per group,
    per-(bs,g) tensor_scalar normalize. This keeps the vector engine well
    under the PE budget (the v1 additive-mask PSUM add was 2x1220ns/chunk).
  - w is PE-transposed per (batch, group); attn@v contracts over entities
    with 2-head column packing; out-projection 4-ki accumulation with fused
    (bias add) x (post-mask mul) epilogue.
  - scheduling: per iteration the 80 QKV matmuls (16 q, 32 k, 32 v) are
    zipped 1:1 with the 80 attention PE ops of the previous iteration
    (16 logit pairs, 16 transposes, 32 attn@v pairs, 16 out-proj), so every
    small-matmul weight load hides under a 512-col k/v stream and the
    vector softmax chain runs ~2.5us ahead of its PE consumers.
  - startup: W_in section DMAs split per-128-col slice and xta per-ki so the
    first q matmul starts as soon as ~192KB has landed; identity/qbd
    memsets issue before any compute.
  - drain: the last iteration defers k(g2=1) and v(b4..7) to act as PE
    fillers for its own attention; out-projection runs per-4-batch subchunk
    with quarter output DMAs so the tail after the last matmul is short.
  - PSUM: 4-buffer ring for projection matmuls, 2 logits banks, 1 transpose
    bank, 1 bank shared by attn@v and out-projection accumulators.
"""
import sys

sys.path.insert(0, "/opt/trn_rl_repo")

import numpy as np
import ml_dtypes

BS, NE, IN_DIM = 1024, 128, 512
EMBED, OUT_DIM = 512, 512
N_HEADS, N_AGENTS = 8, 32
HEAD_DIM = EMBED // N_HEADS  # 64
N_CORES = 8
USE_TTR = False  # fuse mask-mult + row-sum into one tensor_tensor_reduce


def build_nc(b_core: int):
    """Build the per-core Bass program for b_core batches (b_core % 8 == 0)."""
    import concourse.bass as bass
    import concourse.tile as tile
    from concourse import bacc, mybir
    from concourse.masks import make_identity

    F32 = mybir.dt.float32
    BF16 = mybir.dt.bfloat16
    Exp = mybir.ActivationFunctionType.Exp
    Alu = mybir.AluOpType

    assert b_core % 8 == 0
    n_iter = b_core // 8

    nc = bacc.Bacc("TRN2", target_bir_lowering=False, debug=False)

    xt_d = nc.declare_dram_parameter("xt", [b_core, IN_DIM, NE], BF16, isOutput=False)
    xta_d = nc.declare_dram_parameter("xta", [4, 128, b_core, N_AGENTS], BF16, isOutput=False)
    wi_d = nc.declare_dram_parameter("wi", [IN_DIM, 3 * EMBED], BF16, isOutput=False)
    wo_d = nc.declare_dram_parameter("wo", [EMBED, OUT_DIM], BF16, isOutput=False)
    keep_d = nc.declare_dram_parameter("keep", [b_core, N_AGENTS, NE], BF16, isOutput=False)
    pkeep_d = nc.declare_dram_parameter("pkeep", [b_core, N_AGENTS], F32, isOutput=False)
    bias_d = nc.declare_dram_parameter("bias", [OUT_DIM], F32, isOutput=False)
    out_d = nc.declare_dram_parameter("out", [OUT_DIM, b_core, N_AGENTS], BF16, isOutput=True)

    AP = bass.AP

    def dram_ap(handle, offset, ap):
        base = handle[:]
        return AP(tensor=base.tensor, offset=offset, ap=ap)

    with tile.TileContext(nc) as tc:
        with (
            tc.tile_pool(name="const", bufs=1) as constp,
            tc.tile_pool(name="ins", bufs=2) as insp,
            tc.tile_pool(name="mid", bufs=2) as midp,
            tc.tile_pool(name="attn", bufs=2) as attnp,
            tc.tile_pool(name="outs", bufs=2) as outsp,
            tc.tile_pool(name="ps_mm", bufs=4, space="PSUM") as ps_mm,
            tc.tile_pool(name="ps_lg", bufs=1, space="PSUM") as ps_lg,
            tc.tile_pool(name="ps_wt", bufs=1, space="PSUM") as ps_wt,
            tc.tile_pool(name="ps_at", bufs=1, space="PSUM") as ps_at,
        ):
            # ---- constants ----
            wi_sb = constp.tile([128, 4, 3 * EMBED], BF16, name="wi_sb", tag="wi_sb")
            wo_sb = constp.tile([128, 4, OUT_DIM], BF16)
            bias_sb = constp.tile([128, 4], F32)
            ident = constp.tile([128, 128], BF16)
            # block-diagonal q operands, double-buffered manually; off-diagonal
            # zero blocks are written once and never touched again
            qbd = [
                constp.tile([128, 4, 8, 64], BF16, name=f"qbd_{i}", tag=f"qbd_{i}")
                for i in range(2)
            ]

            def emit_wi_section(sec, mo=None):
                # DMA one section (0=q, 1=k, 2=v) of W_in (all 4 ki); per-mo
                # 128-col slices let iter-0 compute start as data arrives
                c0 = sec * EMBED + (0 if mo is None else mo * 128)
                nc2 = EMBED if mo is None else 128
                nc.sync.dma_start(
                    out=wi_sb[:, :, c0 : c0 + nc2],
                    in_=dram_ap(
                        wi_d, c0,
                        [[3 * EMBED, 128], [128 * 3 * EMBED, 4], [1, nc2]],
                    ),
                )

            def emit_early_consts():
                make_identity(nc, ident)
                for i in range(2):
                    nc.gpsimd.memset(qbd[i][0:64, :, :, 32:64], 0.0)
                    nc.gpsimd.memset(qbd[i][64:128, :, :, 0:32], 0.0)

            def emit_late_consts():
                nc.sync.dma_start(
                    out=wo_sb,
                    in_=dram_ap(wo_d, 0, [[OUT_DIM, 128], [128 * OUT_DIM, 4], [1, OUT_DIM]]),
                )
                nc.sync.dma_start(out=bias_sb, in_=dram_ap(bias_d, 0, [[1, 128], [128, 4]]))

            def emit_xta(st, it, ki=None):
                b0 = it * 8
                k0, nk = (0, 4) if ki is None else (ki, 1)
                nc.gpsimd.dma_start(
                    out=st["xta"][:, k0 : k0 + nk, :, :],
                    in_=dram_ap(
                        xta_d,
                        k0 * 128 * b_core * N_AGENTS + b0 * N_AGENTS,
                        [[b_core * N_AGENTS, 128], [128 * b_core * N_AGENTS, nk],
                         [N_AGENTS, 8], [1, N_AGENTS]],
                    ),
                )

            def emit_xt_chunk(st, it, g2, ki):
                b0 = it * 8
                nc.sync.dma_start(
                    out=st["xt"][:, ki, g2 * 4 : (g2 + 1) * 4, :],
                    in_=dram_ap(
                        xt_d,
                        (b0 + g2 * 4) * IN_DIM * NE + ki * 128 * NE,
                        [[NE, 128], [IN_DIM * NE, 4], [1, NE]],
                    ),
                )

            def emit_masks(st, it):
                b0 = it * 8
                # 0/1 keep mask, replicated over the 4 head-pair partition groups
                for cg in range(4):
                    nc.gpsimd.dma_start(
                        out=st["keep"][cg * 32 : (cg + 1) * 32, :, :],
                        in_=dram_ap(
                            keep_d,
                            b0 * N_AGENTS * NE,
                            [[NE, 32], [N_AGENTS * NE, 8], [1, NE]],
                        ),
                    )
                nc.gpsimd.dma_start(
                    out=st["pkeep"],
                    in_=dram_ap(pkeep_d, b0 * N_AGENTS, [[0, 128], [N_AGENTS, 8], [1, N_AGENTS]]),
                )

            def new_state(it):
                st = {"it": it, "qbd": qbd[it % 2]}
                st["xta"] = insp.tile([128, 4, 8, N_AGENTS], BF16, name="xta_sb", tag="xta_sb")
                st["xt"] = insp.tile([128, 4, 8, NE], BF16, name="xt_sb", tag="xt_sb")
                st["keep"] = insp.tile([128, 8, NE], BF16, name="keep_bc", tag="keep_bc")
                # bufs=3: the deferred out-projection reads pkeep(N) during
                # iteration N+2, after iteration N+2's input DMAs are issued
                st["pkeep"] = insp.tile(
                    [128, 8, N_AGENTS], F32, name="pkeep_bc", tag="pkeep_bc", bufs=3
                )
                st["kt"] = midp.tile([128, 4, 8, NE], BF16, name="kt_sb", tag="kt_sb")
                st["vt"] = midp.tile([128, 8, EMBED], BF16, name="vt_sb", tag="vt_sb")
                return st

            def emit_inputs(it):
                """Issue a steady-state iteration's input DMAs."""
                st = new_state(it)
                emit_xta(st, it)
                for g2 in range(2):
                    for ki in range(4):
                        emit_xt_chunk(st, it, g2, ki)
                emit_masks(st, it)
                return st

            # ---- per-iteration fine-grained PE op lists ----

            def q_unit(st, mo):
                """4 q matmuls; returns the PSUM evacuation closure (emitted
                a couple of units later so the copy never heads the scalar/
                vector queue before its data is ready)."""
                q_ps = ps_mm.tile([128, 8, N_AGENTS], F32, tag="mm", name="q_ps")
                for ki in range(4):
                    nc.tensor.matmul(
                        q_ps,
                        wi_sb[:, ki, mo * 128 : (mo + 1) * 128],
                        st["xta"][:, ki, :, :],
                        start=(ki == 0),
                        stop=(ki == 3),
                    )
                qb = st["qbd"]

                def cp():
                    nc.scalar.copy(out=qb[0:64, mo, :, 0:32], in_=q_ps[0:64, :, :])
                    nc.vector.tensor_copy(out=qb[64:128, mo, :, 32:64], in_=q_ps[64:128, :, :])

                return cp

            def k_unit(st, mo, g2):
                k_ps = ps_mm.tile([128, 4, NE], F32, tag="mm", name="k_ps")
                for ki in range(4):
                    nc.tensor.matmul(
                        k_ps,
                        wi_sb[:, ki, EMBED + mo * 128 : EMBED + (mo + 1) * 128],
                        st["xt"][:, ki, g2 * 4 : (g2 + 1) * 4, :],
                        start=(ki == 0),
                        stop=(ki == 3),
                    )

                def cp():
                    if g2 == 0:
                        nc.vector.tensor_copy(
                            out=st["kt"][:, mo, g2 * 4 : (g2 + 1) * 4, :], in_=k_ps
                        )
                    else:
                        nc.scalar.copy(out=st["kt"][:, mo, g2 * 4 : (g2 + 1) * 4, :], in_=k_ps)

                return cp

            def v_unit(st, b):
                v_ps = ps_mm.tile([128, EMBED], F32, tag="mm", name="v_ps")
                for ki in range(4):
                    nc.tensor.matmul(
                        v_ps,
                        st["xt"][:, ki, b, :],
                        wi_sb[:, ki, 2 * EMBED : 3 * EMBED],
                        start=(ki == 0),
                        stop=(ki == 3),
                    )

                def cp():
                    nc.scalar.copy(out=st["vt"][:, b, :], in_=v_ps)

                return cp

            def qkv_units(st, last=False):
                """Unit closures (4 same-weights matmuls each) in v1 order.
                last=True: returns (main12, ktail4, vtail4) with k(g2=1) and
                v(b4..7) deferred for use as drain fillers."""
                qs = [lambda mo=mo: q_unit(st, mo) for mo in range(4)]
                if last:
                    main = qs + [
                        lambda mo=mo: k_unit(st, mo, 0) for mo in range(4)
                    ] + [lambda b=b: v_unit(st, b) for b in range(4)]
                    ktail = [lambda mo=mo: k_unit(st, mo, 1) for mo in range(4)]
                    vtail = [lambda b=b: v_unit(st, b) for b in range(4, 8)]
                    return main, ktail, vtail
                units = list(qs)
                for mo in range(4):
                    units.append(lambda mo=mo: k_unit(st, mo, 0))
                    units.append(lambda mo=mo: k_unit(st, mo, 1))
                    units.append(lambda b=2 * mo: v_unit(st, b))
                    units.append(lambda b=2 * mo + 1: v_unit(st, b))
                return units

            def softmax_ops(st, sc):
                """8 PE closures (logit pairs, g-major) with the scalar/vector
                softmax chain attached: exp per g, then per-(bs,g) mask-mult +
                row-sum (one tensor_tensor_reduce), reciprocal per g,
                per-(bs,g) normalize."""
                qb, kt = st["qbd"], st["kt"]
                lg = [
                    ps_lg.tile([128, 4, NE], F32, tag="lg0", name="lg0"),
                    ps_lg.tile([128, 4, NE], F32, tag="lg1", name="lg1"),
                ]  # [(hp2, h2, a), bs, e] for head-pair-group g = 0, 1
                we = attnp.tile([128, 4, 2, NE], BF16, name="we", tag="we")
                sums = attnp.tile([128, 8], F32, name="sums", tag="sums")
                rcp = attnp.tile([128, 8], F32, name="rcp", tag="rcp")
                wn = attnp.tile([128, 4, 2, NE], BF16, name="wn", tag="wn")
                st[f"wn{sc}"] = wn

                def lg_pair(bs, g):
                    b = sc * 4 + bs
                    for hp2 in range(2):
                        hp = g * 2 + hp2
                        nc.tensor.matmul(
                            lg[g][hp2 * 64 : (hp2 + 1) * 64, bs, :],
                            qb[:, hp, b, :],
                            kt[:, hp, b, :],
                            start=True,
                            stop=True,
                            tile_position=(0, hp2 * 64),
                        )

                def post_g(g):
                    # exp of the whole group's logits (PSUM f32 -> SBUF bf16)
                    nc.scalar.activation(out=we[:, :, g, :], in_=lg[g], func=Exp)
                    # multiplicative 0/1 keep mask + row sums (one op each per g)
                    nc.vector.tensor_mul(
                        wn[:, :, g, :],
                        we[:, :, g, :],
                        st["keep"][:, sc * 4 : (sc + 1) * 4, :],
                    )
                    nc.vector.reduce_sum(
                        sums[:, g * 4 : (g + 1) * 4],
                        wn[:, :, g, :],
                        axis=mybir.AxisListType.X,
                    )
                    nc.vector.reciprocal_approx_fast(
                        out=rcp[:, g * 4 : (g + 1) * 4], in_=sums[:, g * 4 : (g + 1) * 4]
                    )
                    for bs in range(4):
                        nc.vector.tensor_scalar_mul(
                            wn[:, bs, g, :],
                            wn[:, bs, g, :],
                            rcp[:, g * 4 + bs : g * 4 + bs + 1],
                        )

                ops = []
                for g in range(2):
                    for bs in range(4):
                        if bs == 3:
                            ops.append(lambda bs=bs, g=g: (lg_pair(bs, g), post_g(g)))
                        else:
                            ops.append(lambda bs=bs, g=g: lg_pair(bs, g))
                return ops

            def attnv_ops(st, sc, attn_sb):
                """8 transpose closures (g-major, per-(bs,g) PSUM->SBUF copy
                attached) + 16 attn@v pair closures (bs-major, per-bs
                attn_sb copy attached)."""
                vt = st["vt"]
                wt_ps = ps_wt.tile([128, 4, 2, NE], BF16, name="wt_ps")  # [e, bs, g, (hp2,h2,a)]
                wt_sb = attnp.tile([128, 4, 2, NE], BF16, name="wt_sb", tag="wt_sb")
                at_ps = ps_at.tile([128, 4, 4, N_AGENTS], F32, name="at_ps", tag="atop")

                def t_op(bs, g):
                    wn = st[f"wn{sc}"]
                    nc.tensor.transpose(wt_ps[:, bs, g, :], wn[:, bs, g, :], ident)
                    if bs == 3:
                        nc.vector.tensor_copy(out=wt_sb[:, :, g, :], in_=wt_ps[:, :, g, :])

                def av_pair(bs, hp):
                    b = sc * 4 + bs
                    g, hp2 = hp // 2, hp % 2
                    for h2 in range(2):
                        h = hp * 2 + h2
                        nc.tensor.matmul(
                            at_ps[h2 * 64 : h2 * 64 + 64, bs, hp, :],
                            vt[:, b, h * 64 : (h + 1) * 64],
                            wt_sb[:, bs, g, hp2 * 64 + h2 * 32 : hp2 * 64 + (h2 + 1) * 32],
                            start=True,
                            stop=True,
                            tile_position=(0, h2 * 64),
                        )
                    if hp == 3:
                        nc.scalar.copy(
                            out=attn_sb[:, sc * 4 + bs, :, :], in_=at_ps[:, bs, :, :]
                        )

                ops = [lambda bs=bs, g=g: t_op(bs, g) for g in range(2) for bs in range(4)]
                ops += [lambda bs=bs, hp=hp: av_pair(bs, hp) for bs in range(4) for hp in range(4)]
                return ops

            def emit_outdma(st, out_sb, mh, sc=None, queue=None):
                # stream a 256-feature half (or quarter, if sc given) out
                b0, nbs = (0, 8) if sc is None else (sc * 4, 4)
                q = queue if queue is not None else nc.gpsimd
                q.dma_start(
                    out=dram_ap(
                        out_d,
                        mh * 2 * 128 * b_core * N_AGENTS + (st["it"] * 8 + b0) * N_AGENTS,
                        [[b_core * N_AGENTS, 128],
                         [128 * b_core * N_AGENTS, 2],
                         [N_AGENTS, nbs],
                         [1, N_AGENTS]],
                    ),
                    in_=out_sb[:, mh * 2 : mh * 2 + 2, b0 : b0 + nbs, :],
                )

            def outproj_ops(st, attn_sb, out_sb, sc, dma, queue=None):
                """Closures for the out-projection: 16 matmuls with the
                (bias add) x (post-mask mul) epilogue + output DMA attached."""
                bs0, nbs = (0, 8) if sc is None else (sc * 4, 4)
                # cell(mh1) lives in the wt bank (free by out-projection time):
                # with both cells in one bank, mh1's matmuls stall ~2.9us on
                # mh0's epilogue stt every iteration
                cell = {
                    0: ps_at.tile([128, 2, nbs, N_AGENTS], F32, name="op_ps0", tag="atop"),
                    1: ps_lg.tile([128, 2, nbs, N_AGENTS], F32, name="op_ps1", tag="lg0"),
                }

                def mm(mh, m2, ki2):
                    mo2 = mh * 2 + m2
                    nc.tensor.matmul(
                        cell[mh][:, m2, :, :],
                        wo_sb[:, ki2, mo2 * 128 : (mo2 + 1) * 128],
                        attn_sb[:, bs0 : bs0 + nbs, ki2, :],
                        start=(ki2 == 0),
                        stop=(ki2 == 3),
                    )

                def stt(mh):
                    for m2 in range(2):
                        mo2 = mh * 2 + m2
                        nc.vector.scalar_tensor_tensor(
                            out=out_sb[:, mo2, bs0 : bs0 + nbs, :],
                            in0=cell[mh][:, m2, :, :],
                            scalar=bias_sb[:, mo2 : mo2 + 1],
                            in1=st["pkeep"][:, bs0 : bs0 + nbs, :],
                            op0=Alu.add,
                            op1=Alu.mult,
                        )
                    if dma:
                        emit_outdma(st, out_sb, mh, sc=sc, queue=queue)

                # the epilogue stt closures come after ALL matmuls so the
                # vector queue never blocks on a not-yet-computed cell
                ops = [
                    lambda mh=mh, m2=m2, ki2=ki2: mm(mh, m2, ki2)
                    for mh in range(2) for m2 in range(2) for ki2 in range(4)
                ]
                ops.append(lambda: stt(0))
                ops.append(lambda: stt(1))
                return ops

            def attn_blocks(st):
                """One iteration's attention as 5 contiguous PE blocks
                [sm0, av0, sm1, av1, op]. Blocks are popped whole between qkv
                units: every block boundary costs ~100ns of exposed weight
                load on the next unit lead, so fewer boundaries win."""
                attn_sb = outsp.tile([128, 8, 4, N_AGENTS], BF16, name="attn_sb", tag="attn_sb")
                out_sb = outsp.tile([128, 4, 8, N_AGENTS], BF16, name="out_sb", tag="out_sb")
                st["attn_sb"] = attn_sb
                st["out_sb"] = out_sb
                sm0 = softmax_ops(st, 0)
                sm1 = softmax_ops(st, 1)
                av0 = attnv_ops(st, 0, attn_sb)
                av1 = attnv_ops(st, 1, attn_sb)
                opl = outproj_ops(st, attn_sb, out_sb, None, dma=True)
                return [sm0, av0, sm1, av1, opl]

            # ---- head: iteration 0 with DMA-overlapped startup ----
            st0 = new_state(0)
            emit_wi_section(0, mo=0)
            emit_xta(st0, 0, ki=0)
            emit_xta(st0, 0, ki=1)
            emit_xta(st0, 0, ki=2)
            emit_xta(st0, 0, ki=3)
            emit_early_consts()
            emit_wi_section(0, mo=1)
            emit_wi_section(0, mo=2)
            emit_wi_section(0, mo=3)
            for ki in range(4):
                emit_xt_chunk(st0, 0, 0, ki)
                emit_wi_section(1, mo=ki)
            for ki in range(4):
                emit_xt_chunk(st0, 0, 1, ki)
                emit_wi_section(2, mo=ki)
            emit_masks(st0, 0)
            emit_late_consts()

            # pops per unit: logits pre-pop before the units (their vector
            # softmax chain then runs ahead of the q-copy burst); transposes
            # from u4 (wn ready ~u3), attn@v after the wt copies, out-proj
            # once sc1's attn_sb lands.
            # block pop positions (attention block b emitted after unit index)
            STEADY_AT = {3: 0, 7: 1, 11: 2, 15: 3, 17: 4}
            LAST_AT = {3: 0, 5: 1, 7: 2, 9: 3, 11: 4}

            # evacuation copies are flushed one unit after their matmuls so
            # they never head an engine queue before their data is ready;
            # the deque carries across iterations (non-PE, so flushing between
            # units does not break PE weight-load pipelining)
            useq = [0]
            pend = []

            def run_unit(u):
                cp = u()
                while pend and pend[0][0] <= useq[0] - 1:
                    pend.pop(0)[1]()
                pend.append((useq[0], cp))
                useq[0] += 1

            def flush_pend():
                while pend:
                    pend.pop(0)[1]()

            def weave(units, blocks, at):
                for i, u in enumerate(units):
                    run_unit(u)
                    if i in at:
                        for op in blocks[at[i]]:
                            op()

            prev = None
            for it in range(n_iter - 1):
                st = st0 if it == 0 else emit_inputs(it)
                units = qkv_units(st)
                if prev is not None:
                    weave(units, attn_blocks(prev), STEADY_AT)
                else:
                    for u in units:
                        run_unit(u)
                prev = st

            # ---- last iteration: weave prev's attention with main qkv, then
            # self-interleave its own attention with the deferred k/v fillers
            st = emit_inputs(n_iter - 1) if n_iter > 1 else st0
            main, ktail, vtail = qkv_units(st, last=True)
            if prev is not None:
                weave(main, attn_blocks(prev), LAST_AT)
            else:
                for u in main:
                    run_unit(u)

            attn_sb = outsp.tile([128, 8, 4, N_AGENTS], BF16, name="attn_sb", tag="attn_sb")
            out_sb = outsp.tile([128, 4, 8, N_AGENTS], BF16, name="out_sb", tag="out_sb")
            st["attn_sb"] = attn_sb
            sm0 = softmax_ops(st, 0)
            for u in ktail:  # 4 k units; 8 logit pairs woven in
                run_unit(u)
                sm0.pop(0)()
                sm0.pop(0)()
            av0 = attnv_ops(st, 0, attn_sb)
            for i, u in enumerate(vtail):  # 4 v units; 24 attn ops woven in
                run_unit(u)
                for _ in range(6):
                    av0.pop(0)()
            flush_pend()
            sm1 = softmax_ops(st, 1)
            op0 = outproj_ops(st, attn_sb, out_sb, 0, dma=True)
            for i, u in enumerate(op0):  # 16 op matmuls; 8 logit pairs woven in
                u()
                if i % 2 == 1 and sm1:
                    sm1.pop(0)()
            for u in attnv_ops(st, 1, attn_sb):
                u()
            for u in outproj_ops(st, attn_sb, out_sb, 1, dma=True, queue=nc.sync):
                u()

    nc.compile()
    return nc


def _prep_core_inputs(ents, keep, pkeep, wi, wo, bias):
    """Host-side layout prep for one core's batch shard."""
    b_core = ents.shape[0]
    xt = np.ascontiguousarray(ents.transpose(0, 2, 1))  # [b, in, e]
    xta = np.ascontiguousarray(
        ents[:, :N_AGENTS, :].transpose(2, 0, 1)
    ).reshape(4, 128, b_core, N_AGENTS)
    return {
        "xt": xt,
        "xta": xta,
        "wi": wi,
        "wo": wo,
        "keep": keep,
        "pkeep": pkeep,
        "bias": bias,
    }


def run(entities, pre_mask, post_mask, W_in, W_out, b_out, trace=False):
    """Shard, run on 8 cores, gather. Returns (out, BassKernelResults)."""
    from concourse.bass_utils import run_bass_kernel_spmd

    bs = entities.shape[0]
    b_core = bs // N_CORES
    entities = np.asarray(entities, dtype=np.float32).astype(ml_dtypes.bfloat16)
    keep = (~np.asarray(pre_mask)).astype(ml_dtypes.bfloat16)
    pkeep = (~np.asarray(post_mask)).astype(np.float32)
    wi_f = np.ascontiguousarray(np.asarray(W_in, dtype=np.float32).T)
    wi_f[:, :EMBED] *= np.float32(0.125)  # fold the 1/sqrt(head_dim) into W_q
    wi = wi_f.astype(ml_dtypes.bfloat16)
    wo = np.ascontiguousarray(np.asarray(W_out, dtype=np.float32).T).astype(ml_dtypes.bfloat16)
    bias = np.asarray(b_out, dtype=np.float32)

    nc = build_nc(b_core)
    in_maps = [
        _prep_core_inputs(
            entities[c * b_core : (c + 1) * b_core],
            keep[c * b_core : (c + 1) * b_core],
            pkeep[c * b_core : (c + 1) * b_core],
            wi, wo, bias,
        )
        for c in range(N_CORES)
    ]
    res = run_bass_kernel_spmd(nc, in_maps, list(range(N_CORES)), trace=trace)
    out = np.empty((bs, N_AGENTS, OUT_DIM), dtype=np.float32)
    for c in range(N_CORES):
        out[c * b_core : (c + 1) * b_core] = (
            res.results[c]["out"].astype(np.float32).transpose(1, 2, 0)
        )
    return out, res


def kernel(entities, pre_mask, post_mask, W_in, W_out, b_out):
    out, _ = run(entities, pre_mask, post_mask, W_in, W_out, b_out, trace=False)
    return out
